# revision 23
# baseline (speedup 1.0000x reference)
"""Multi-head self-attention (B=4, S=2048, D=1024, H=16, causal) on 8 TRN2 cores.

Sharding: core = (batch b, head-group g) with b = core//2, g = core%2.
Each core computes Q/K/V projections for its batch restricted to its 8 heads
(column-parallel), causal flash attention for those heads, and a row-parallel
partial of the output projection. Host sums the two partials per batch and
adds the bias terms. Zero collectives; every core runs the identical program
on different data.

On-device layout is feature-on-partition ("transposed"): qT/kT [e, t] so the
scores matmul produces sT [k, q] tiles directly (no PE transposes). Softmax
skips max-subtraction (scores ~ N(0,1) after the 1/sqrt(dh) fold into Wq, so
fp32 exp is safe). The softmax denominator comes for free as a 65th "ones"
row in the AV matmul. All matmuls run in float32r (full fp32 storage,
bf16-rate PE, ~13-bit mantissa).
"""

import numpy as np

B = 4
S = 2048
D = 1024
H = 16
DH = 64
HG = 8            # heads per core
E = HG * DH       # 512 features per head-group
P = 128
NCORES = 8

DC = D // P       # 8 d-chunks
EC = E // P       # 4 e-chunks per group
TC4 = S // 512    # 4 token 512-chunks
TT = S // P       # 16 token 128-tiles
QB = S // 512     # 4 query blocks of 512
NEG = -1.0e9

_CACHE = {}


def _build_nc(phases=(1, 2, 3)):
    import concourse.mybir as mybir
    from concourse import bacc
    from concourse.tile import TileContext

    f32 = mybir.dt.float32
    f32r = mybir.dt.float32r

    nc = bacc.Bacc("TRN2", target_bir_lowering=False, name="mhsa")
    xT = nc.dram_tensor("xT", [D, S], f32r, kind="ExternalInput")
    wq = nc.dram_tensor("wq", [D, E], f32r, kind="ExternalInput")
    wk = nc.dram_tensor("wk", [D, E], f32r, kind="ExternalInput")
    wv = nc.dram_tensor("wv", [D, E], f32r, kind="ExternalInput")
    wo = nc.dram_tensor("wo", [E, D], f32r, kind="ExternalInput")
    bq = nc.dram_tensor("bq", [P, EC], f32, kind="ExternalInput")
    bk = nc.dram_tensor("bk", [P, EC], f32, kind="ExternalInput")
    cm = nc.dram_tensor("cm", [P, 4, 512], f32, kind="ExternalInput")
    outp = nc.dram_tensor("outp", [S, D], f32, kind="ExternalOutput")

    with TileContext(nc) as tc:
        with tc.tile_pool(name="persist", bufs=1) as persist, \
             tc.tile_pool(name="dram", bufs=4, space="DRAM") as dram_pool:

            qT_all = persist.tile([P, EC, S], f32r)      # 4 MB
            kT_all = persist.tile([P, EC, S], f32r)      # 4 MB
            v_aug = persist.tile([P, TT, HG, DH + 1], f32r)  # ~4.3 MB
            cT_all = persist.tile([P, EC, S], f32r)      # 4 MB
            cm_sb = persist.tile([P, 4, 512], f32)       # 1 MB
            bq_sb = persist.tile([P, EC], f32)
            bk_sb = persist.tile([P, EC], f32)
            nc.sync.dma_start(cm_sb, cm.ap())
            nc.sync.dma_start(bq_sb, bq.ap())
            nc.sync.dma_start(bk_sb, bk.ap())

            # ones columns of v_aug
            ones_f32 = persist.tile([P, TT, HG], f32)
            nc.vector.memset(ones_f32, 1.0)
            nc.vector.tensor_copy(v_aug[:, :, :, DH], ones_f32)

            # ---------------- Phase 1: projections ----------------
            TCH = 256                     # token chunk (f32r full rate at N>=256)
            NCH = S // TCH
            with tc.tile_pool(name="wpool", bufs=1) as wpool, \
                 tc.tile_pool(name="xpool", bufs=2) as xpool, \
                 tc.tile_pool(name="ps_proj", bufs=4, space="PSUM") as ps_proj:
                # Q and K in transposed layout qT[e, t]; one shared xT pass
                if 1 in phases:
                    # wq in per-ec tiles so the first matmul starts after a
                    # 0.5 MB load instead of the full 6 MB of weights
                    wq_sbs = []
                    for ec in range(EC):
                        t = wpool.tile([P, DC, P], f32r, tag=f"wq{ec}")
                        nc.sync.dma_start(
                            t, wq[:, ec * P:(ec + 1) * P]
                            .rearrange("(dc p) e -> p dc e", p=P))
                        wq_sbs.append(t)
                    wk_sb = wpool.tile([P, DC, E], f32r, tag="wk")
                    nc.sync.dma_start(
                        wk_sb, wk.rearrange("(dc p) e -> p dc e", p=P))
                if 1 in phases:
                    wv_sb = wpool.tile([P, DC, E], f32r, tag="wv")
                    nc.sync.dma_start(
                        wv_sb, wv.rearrange("(dc p) e -> p dc e", p=P))
                for t4 in (range(NCH) if 1 in phases else ()):
                    ts_ = slice(t4 * TCH, (t4 + 1) * TCH)
                    xt = xpool.tile([P, DC, TCH], f32r, tag="xt")
                    nc.sync.dma_start(
                        xt, xT[:, ts_].rearrange("(dc p) t -> p dc t", p=P))
                    for wsel, b_sb, dst in (("q", bq_sb, qT_all),
                                            ("k", bk_sb, kT_all)):
                        for ec in range(EC):
                            ps = ps_proj.tile([P, TCH], mybir.dt.float32,
                                              tag="pp")
                            for dc in range(DC):
                                lhsT = (wq_sbs[ec][:, dc] if wsel == "q"
                                        else wk_sb[:, dc, ec * P:(ec + 1) * P])
                                nc.tensor.matmul(
                                    ps, lhsT, xt[:, dc],
                                    start=(dc == 0), stop=(dc == DC - 1))
                            nc.scalar.activation(
                                dst[:, ec, ts_], ps,
                                mybir.ActivationFunctionType.Identity,
                                bias=b_sb[:, ec:ec + 1])
                    # V in natural layout v[t, e] (+ ones col); bv on host
                    for tb in range(TCH // P):
                        ps = ps_proj.tile([P, E], mybir.dt.float32, tag="pp")
                        for dc in range(DC):
                            nc.tensor.matmul(
                                ps, xt[:, dc, tb * P:(tb + 1) * P], wv_sb[:, dc],
                                start=(dc == 0), stop=(dc == DC - 1))
                        tt = t4 * (TCH // P) + tb
                        nc.vector.tensor_copy(
                            v_aug[:, tt, :, 0:DH],
                            ps.rearrange("p (h d) -> p h d", h=HG))

            # ---------- Phases 2+3: attention + fused out-projection ----
            # qb outermost so each q-block's out-projection overlaps the next
            # q-block's attention. Diagonal k-tiles are processed ragged:
            # only columns >= DSTART[j] (min width 256 keeps f32r full-rate).
            DSTART = (0, 128, 256, 256)
            with tc.tile_pool(name="ptpool", bufs=6) as ptpool, \
                 tc.tile_pool(name="normpool", bufs=4) as normpool, \
                 tc.tile_pool(name="wopool", bufs=1) as wopool, \
                 tc.tile_pool(name="evict", bufs=2) as evict, \
                 tc.tile_pool(name="ps_s", bufs=2, space="PSUM") as ps_s_pool, \
                 tc.tile_pool(name="ps_av", bufs=2, space="PSUM") as ps_av_pool, \
                 tc.tile_pool(name="ps_o", bufs=2, space="PSUM") as ps_o_pool:
                wo_sb = wopool.tile([P, EC, D], f32r, tag="wo")
                if 2 in phases or 3 in phases:
                    nc.sync.dma_start(
                        wo_sb, wo.rearrange("(dc p) e -> p dc e", p=P))
                for qb in (range(QB) if 2 in phases else ()):
                    nkt = qb * 4 + 4      # causal k-tiles for this q block
                    q0 = qb * 512
                    for hp in range(EC):  # head pair chunk: heads 2hp, 2hp+1
                        ps_av0 = ps_av_pool.tile([DH + 1, 512],
                                                 mybir.dt.float32, tag="av")
                        ps_av1 = ps_av_pool.tile([DH + 1, 512],
                                                 mybir.dt.float32, tag="av")
                        qs = slice(q0, q0 + 512)
                        for kt in range(nkt):
                            ks = slice(kt * P, (kt + 1) * P)
                            diag = kt >= qb * 4
                            d0 = DSTART[kt - qb * 4] if diag else 0
                            w = 512 - d0
                            qsd = slice(q0 + d0, q0 + 512)
                            # both heads' scores in one 2-bank psum tile
                            ps_s = ps_s_pool.tile([P, 2, 512],
                                                  mybir.dt.float32, tag="s")
                            nc.tensor.matmul(ps_s[:, 0, d0:],
                                             kT_all[0:DH, hp, ks],
                                             qT_all[0:DH, hp, qsd],
                                             start=True, stop=True,
                                             tile_position=(0, 0))
                            nc.tensor.matmul(ps_s[:, 1, d0:],
                                             kT_all[DH:P, hp, ks],
                                             qT_all[DH:P, hp, qsd],
                                             start=True, stop=True,
                                             tile_position=(64, 0))
                            pt = ptpool.tile([P, 2, 512], f32r, tag="pt")
                            # exp straight from psum (single op for 2 heads);
                            # diag masking is multiplicative afterwards, on
                            # SBUF (DVE 2x mode), off the ACT input path
                            nc.scalar.activation(
                                pt[:, :, d0:], ps_s[:, :, d0:],
                                mybir.ActivationFunctionType.Exp)
                            if diag:
                                j = kt - qb * 4
                                nc.vector.tensor_tensor(
                                    pt[:, :, d0:], pt[:, :, d0:],
                                    cm_sb[:, j, None, d0:]
                                    .to_broadcast([P, 2, w]),
                                    mybir.AluOpType.mult)
                            nc.tensor.matmul(
                                ps_av0[:, d0:], v_aug[:, kt, 2 * hp],
                                pt[:, 0, d0:],
                                start=(kt == 0), stop=(kt == nkt - 1))
                            nc.tensor.matmul(
                                ps_av1[:, d0:], v_aug[:, kt, 2 * hp + 1],
                                pt[:, 1, d0:],
                                start=(kt == 0), stop=(kt == nkt - 1))
                        # evict AV psum to SBUF fast (frees the bank), then
                        # normalize off the critical path:
                        # c = av[0:64] * (1/av[64]) broadcast over rows
                        for idx, ps_av in ((0, ps_av0), (1, ps_av1)):
                            av_sb = normpool.tile([DH + 1, 512], f32, tag="avs")
                            nc.vector.tensor_copy(av_sb, ps_av)
                            recip = normpool.tile([1, 512], f32, tag="recip")
                            nc.vector.reciprocal(recip, av_sb[DH:DH + 1])
                            r_dram = dram_pool.tile([1, 512], f32, tag="rd")
                            nc.sync.dma_start(r_dram, recip)
                            r_rep = normpool.tile([DH, 512], f32, tag="rrep")
                            nc.sync.dma_start(
                                r_rep, r_dram.to_broadcast([DH, 512]))
                            nc.vector.tensor_tensor(
                                cT_all[idx * DH:(idx + 1) * DH, hp, qs],
                                av_sb[0:DH], r_rep, mybir.AluOpType.mult)
                    # partial out-projection, delayed one q-block so the
                    # preceding normalize chain stays off the critical path
                    if 3 in phases:
                        for tb in (range((qb - 1) * 4, qb * 4) if qb > 0
                                   else ()):
                            for eb in range(2):
                                ps = ps_o_pool.tile([P, 512], mybir.dt.float32,
                                                    tag="pp")
                                for dc in range(EC):
                                    nc.tensor.matmul(
                                        ps, cT_all[:, dc, tb * P:(tb + 1) * P],
                                        wo_sb[:, dc, eb * 512:(eb + 1) * 512],
                                        start=(dc == 0), stop=(dc == EC - 1))
                                o_sb = evict.tile([P, 512], f32, tag="o")
                                nc.vector.tensor_copy(o_sb, ps)
                                nc.sync.dma_start(
                                    outp.ap()[tb * P:(tb + 1) * P,
                                              eb * 512:(eb + 1) * 512],
                                    o_sb)
                # trailing q-block's out-projection
                for tb in (range((QB - 1) * 4, QB * 4)
                           if (2 in phases and 3 in phases) else ()):
                    for eb in range(2):
                        ps = ps_o_pool.tile([P, 512], mybir.dt.float32,
                                            tag="pp")
                        for dc in range(EC):
                            nc.tensor.matmul(
                                ps, cT_all[:, dc, tb * P:(tb + 1) * P],
                                wo_sb[:, dc, eb * 512:(eb + 1) * 512],
                                start=(dc == 0), stop=(dc == EC - 1))
                        o_sb = evict.tile([P, 512], f32, tag="o")
                        nc.vector.tensor_copy(o_sb, ps)
                        nc.sync.dma_start(
                            outp.ap()[tb * P:(tb + 1) * P,
                                      eb * 512:(eb + 1) * 512],
                            o_sb)

    nc.finalize()
    return nc


def make_in_maps(x, Wq, bq, Wk, bk, Wv, bv, Wo, bo, mask):
    """Build the 8 per-core input dicts (host-side shard + transform)."""
    x = np.asarray(x, dtype=np.float32)
    Wq = np.asarray(Wq, dtype=np.float32)
    Wk = np.asarray(Wk, dtype=np.float32)
    Wv = np.asarray(Wv, dtype=np.float32)
    Wo = np.asarray(Wo, dtype=np.float32)
    bqf = np.asarray(bq, dtype=np.float32)
    bkf = np.asarray(bk, dtype=np.float32)
    mask = np.asarray(mask)

    scale = 1.0 / np.sqrt(np.float32(DH))
    # torch convention y = x @ W.T: feed W.T with d_in on axis 0
    WqT = np.ascontiguousarray(Wq.T) * scale        # [D, D], scale folded
    WkT = np.ascontiguousarray(Wk.T)
    WvT = np.ascontiguousarray(np.asarray(Wv, dtype=np.float32).T)
    WoT = np.ascontiguousarray(Wo.T)                # [D, D]

    # causal diag bias tiles from the mask input: tile j covers keys
    # [q0+128j, q0+128j+128) for query block [q0, q0+512); tril is
    # translation-invariant so build from q0 = S-512.
    q0 = S - 512
    m2 = mask.reshape(S, S)
    cm = np.empty((P, 4, 512), np.float32)
    for j in range(4):
        sub = m2[q0:q0 + 512, q0 + 128 * j:q0 + 128 * j + 128]  # [q, k]
        cm[:, j, :] = np.where(sub.T != 0, 1.0, 0.0)

    in_maps = []
    for core in range(NCORES):
        b, g = divmod(core, 2)
        cols = slice(g * E, (g + 1) * E)
        in_maps.append({
            "xT": np.ascontiguousarray(x[b].T),          # [D, S]
            "wq": np.ascontiguousarray(WqT[:, cols]),
            "wk": np.ascontiguousarray(WkT[:, cols]),
            "wv": np.ascontiguousarray(WvT[:, cols]),
            "wo": np.ascontiguousarray(WoT[cols, :]),
            "bq": np.ascontiguousarray((bqf[cols] * scale).reshape(EC, P).T),
            "bk": np.ascontiguousarray(bkf[cols].reshape(EC, P).T),
            "cm": cm,
        })
    return in_maps


def assemble_output(results, bv, bo, Wo):
    """Sum per-batch partials and add the bias correction."""
    bv = np.asarray(bv, dtype=np.float32)
    bo = np.asarray(bo, dtype=np.float32)
    Wo = np.asarray(Wo, dtype=np.float32)
    # context bias bv contributes bv @ Wo.T (attn rows sum to 1)
    corr = (bo + bv @ Wo.T).astype(np.float32)      # [D]
    out = np.empty((B, S, D), np.float32)
    for b in range(B):
        out[b] = results[2 * b]["outp"] + results[2 * b + 1]["outp"] + corr
    return out


def kernel(x, Wq, bq, Wk, bk, Wv, bv, Wo, bo, mask):
    from concourse.bass_utils import run_bass_kernel_spmd

    if "nc" not in _CACHE:
        _CACHE["nc"] = _build_nc()
    nc = _CACHE["nc"]
    in_maps = make_in_maps(x, Wq, bq, Wk, bk, Wv, bv, Wo, bo, mask)
    res = run_bass_kernel_spmd(nc, in_maps, core_ids=list(range(NCORES)))
    return assemble_output(res.results, bv, bo, Wo)


# revision 26
# speedup vs baseline: 1.0070x; 1.0070x over previous
"""Multi-head self-attention (B=4, S=2048, D=1024, H=16, causal) on 8 TRN2 cores.

Sharding: core = (batch b, head-group g) with b = core//2, g = core%2.
Each core computes Q/K/V projections for its batch restricted to its 8 heads
(column-parallel), causal flash attention for those heads, and a row-parallel
partial of the output projection. Host sums the two partials per batch and
adds the bias terms. Zero collectives; every core runs the identical program
on different data.

On-device layout is feature-on-partition ("transposed"): qT/kT [e, t] so the
scores matmul produces sT [k, q] tiles directly (no PE transposes). Softmax
skips max-subtraction (scores ~ N(0,1) after the 1/sqrt(dh) fold into Wq, so
fp32 exp is safe). The softmax denominator comes for free as a 65th "ones"
row in the AV matmul. All matmuls run in float32r (full fp32 storage,
bf16-rate PE, ~13-bit mantissa).
"""

import numpy as np

B = 4
S = 2048
D = 1024
H = 16
DH = 64
HG = 8            # heads per core
E = HG * DH       # 512 features per head-group
P = 128
NCORES = 8

DC = D // P       # 8 d-chunks
EC = E // P       # 4 e-chunks per group
TC4 = S // 512    # 4 token 512-chunks
TT = S // P       # 16 token 128-tiles
QB = S // 512     # 4 query blocks of 512
NEG = -1.0e9

_CACHE = {}


def _build_nc(phases=(1, 2, 3)):
    import concourse.mybir as mybir
    from concourse import bacc
    from concourse.tile import TileContext

    f32 = mybir.dt.float32
    f32r = mybir.dt.float32r

    nc = bacc.Bacc("TRN2", target_bir_lowering=False, name="mhsa")
    xT = nc.dram_tensor("xT", [D, S], f32r, kind="ExternalInput")
    wq = nc.dram_tensor("wq", [D, E], f32r, kind="ExternalInput")
    wk = nc.dram_tensor("wk", [D, E], f32r, kind="ExternalInput")
    wv = nc.dram_tensor("wv", [D, E], f32r, kind="ExternalInput")
    wo = nc.dram_tensor("wo", [E, D], f32r, kind="ExternalInput")
    bq = nc.dram_tensor("bq", [P, EC], f32, kind="ExternalInput")
    bk = nc.dram_tensor("bk", [P, EC], f32, kind="ExternalInput")
    cm = nc.dram_tensor("cm", [P, 4, 512], f32, kind="ExternalInput")
    outp = nc.dram_tensor("outp", [S, D], f32, kind="ExternalOutput")

    with TileContext(nc) as tc:
        with tc.tile_pool(name="persist", bufs=1) as persist, \
             tc.tile_pool(name="dram", bufs=4, space="DRAM") as dram_pool:

            qT_all = persist.tile([P, EC, S], f32r)      # 4 MB
            kT_all = persist.tile([P, EC, S], f32r)      # 4 MB
            v_aug = persist.tile([P, TT, HG, DH + 1], f32r)  # ~4.3 MB
            cT_all = persist.tile([P, EC, S], f32r)      # 4 MB
            cm_sb = persist.tile([P, 4, 512], f32)       # 1 MB
            bq_sb = persist.tile([P, EC], f32)
            bk_sb = persist.tile([P, EC], f32)
            nc.sync.dma_start(cm_sb, cm.ap())
            nc.sync.dma_start(bq_sb, bq.ap())
            nc.sync.dma_start(bk_sb, bk.ap())

            # ones columns of v_aug
            ones_f32 = persist.tile([P, TT, HG], f32)
            nc.vector.memset(ones_f32, 1.0)
            nc.vector.tensor_copy(v_aug[:, :, :, DH], ones_f32)

            # ---------------- Phase 1: projections ----------------
            TCH = 256                     # token chunk (f32r full rate at N>=256)
            NCH = S // TCH
            with tc.tile_pool(name="wpool", bufs=1) as wpool, \
                 tc.tile_pool(name="xpool", bufs=2) as xpool, \
                 tc.tile_pool(name="ps_proj", bufs=4, space="PSUM") as ps_proj:
                # Q and K in transposed layout qT[e, t]; one shared xT pass
                if 1 in phases:
                    # wq in per-ec tiles so the first matmul starts after a
                    # 0.5 MB load instead of the full 6 MB of weights
                    wq_sbs = []
                    for ec in range(EC):
                        t = wpool.tile([P, DC, P], f32r, tag=f"wq{ec}")
                        nc.sync.dma_start(
                            t, wq[:, ec * P:(ec + 1) * P]
                            .rearrange("(dc p) e -> p dc e", p=P))
                        wq_sbs.append(t)
                    wk_sb = wpool.tile([P, DC, E], f32r, tag="wk")
                    nc.sync.dma_start(
                        wk_sb, wk.rearrange("(dc p) e -> p dc e", p=P))
                if 1 in phases:
                    wv_sb = wpool.tile([P, DC, E], f32r, tag="wv")
                    nc.sync.dma_start(
                        wv_sb, wv.rearrange("(dc p) e -> p dc e", p=P))
                for t4 in (range(NCH) if 1 in phases else ()):
                    ts_ = slice(t4 * TCH, (t4 + 1) * TCH)
                    xt = xpool.tile([P, DC, TCH], f32r, tag="xt")
                    for dc in range(DC):
                        nc.sync.dma_start(
                            xt[:, dc], xT[dc * P:(dc + 1) * P, ts_])
                    for wsel, b_sb, dst in (("q", bq_sb, qT_all),
                                            ("k", bk_sb, kT_all)):
                        for ec in range(EC):
                            ps = ps_proj.tile([P, TCH], mybir.dt.float32,
                                              tag="pp")
                            for dc in range(DC):
                                lhsT = (wq_sbs[ec][:, dc] if wsel == "q"
                                        else wk_sb[:, dc, ec * P:(ec + 1) * P])
                                nc.tensor.matmul(
                                    ps, lhsT, xt[:, dc],
                                    start=(dc == 0), stop=(dc == DC - 1))
                            nc.scalar.activation(
                                dst[:, ec, ts_], ps,
                                mybir.ActivationFunctionType.Identity,
                                bias=b_sb[:, ec:ec + 1])
                    # V in natural layout v[t, e] (+ ones col); bv on host
                    for tb in range(TCH // P):
                        ps = ps_proj.tile([P, E], mybir.dt.float32, tag="pp")
                        for dc in range(DC):
                            nc.tensor.matmul(
                                ps, xt[:, dc, tb * P:(tb + 1) * P], wv_sb[:, dc],
                                start=(dc == 0), stop=(dc == DC - 1))
                        tt = t4 * (TCH // P) + tb
                        nc.vector.tensor_copy(
                            v_aug[:, tt, :, 0:DH],
                            ps.rearrange("p (h d) -> p h d", h=HG))

            # ---------- Phases 2+3: attention + fused out-projection ----
            # qb outermost so each q-block's out-projection overlaps the next
            # q-block's attention. Diagonal k-tiles are processed ragged:
            # only columns >= DSTART[j] (min width 256 keeps f32r full-rate).
            DSTART = (0, 128, 256, 256)
            with tc.tile_pool(name="ptpool", bufs=6) as ptpool, \
                 tc.tile_pool(name="normpool", bufs=4) as normpool, \
                 tc.tile_pool(name="wopool", bufs=1) as wopool, \
                 tc.tile_pool(name="evict", bufs=3) as evict, \
                 tc.tile_pool(name="ps_s", bufs=2, space="PSUM") as ps_s_pool, \
                 tc.tile_pool(name="ps_av", bufs=2, space="PSUM") as ps_av_pool, \
                 tc.tile_pool(name="ps_o", bufs=2, space="PSUM") as ps_o_pool:
                wo_sb = wopool.tile([P, EC, D], f32r, tag="wo")
                if 2 in phases or 3 in phases:
                    nc.sync.dma_start(
                        wo_sb, wo.rearrange("(dc p) e -> p dc e", p=P))
                for qb in (range(QB) if 2 in phases else ()):
                    nkt = qb * 4 + 4      # causal k-tiles for this q block
                    q0 = qb * 512
                    for hp in range(EC):  # head pair chunk: heads 2hp, 2hp+1
                        ps_av0 = ps_av_pool.tile([DH + 1, 512],
                                                 mybir.dt.float32, tag="av")
                        ps_av1 = ps_av_pool.tile([DH + 1, 512],
                                                 mybir.dt.float32, tag="av")
                        qs = slice(q0, q0 + 512)
                        for kt in range(nkt):
                            ks = slice(kt * P, (kt + 1) * P)
                            diag = kt >= qb * 4
                            d0 = DSTART[kt - qb * 4] if diag else 0
                            w = 512 - d0
                            qsd = slice(q0 + d0, q0 + 512)
                            # both heads' scores in one 2-bank psum tile
                            ps_s = ps_s_pool.tile([P, 2, 512],
                                                  mybir.dt.float32, tag="s")
                            nc.tensor.matmul(ps_s[:, 0, d0:],
                                             kT_all[0:DH, hp, ks],
                                             qT_all[0:DH, hp, qsd],
                                             start=True, stop=True,
                                             tile_position=(0, 0))
                            nc.tensor.matmul(ps_s[:, 1, d0:],
                                             kT_all[DH:P, hp, ks],
                                             qT_all[DH:P, hp, qsd],
                                             start=True, stop=True,
                                             tile_position=(64, 0))
                            pt = ptpool.tile([P, 2, 512], f32r, tag="pt")
                            # exp straight from psum (single op for 2 heads);
                            # diag masking is multiplicative afterwards, on
                            # SBUF (DVE 2x mode), off the ACT input path
                            nc.scalar.activation(
                                pt[:, :, d0:], ps_s[:, :, d0:],
                                mybir.ActivationFunctionType.Exp)
                            if diag:
                                j = kt - qb * 4
                                nc.vector.tensor_tensor(
                                    pt[:, :, d0:], pt[:, :, d0:],
                                    cm_sb[:, j, None, d0:]
                                    .to_broadcast([P, 2, w]),
                                    mybir.AluOpType.mult)
                            nc.tensor.matmul(
                                ps_av0[:, d0:], v_aug[:, kt, 2 * hp],
                                pt[:, 0, d0:],
                                start=(kt == 0), stop=(kt == nkt - 1))
                            nc.tensor.matmul(
                                ps_av1[:, d0:], v_aug[:, kt, 2 * hp + 1],
                                pt[:, 1, d0:],
                                start=(kt == 0), stop=(kt == nkt - 1))
                        # evict AV psum to SBUF fast (frees the bank), then
                        # normalize off the critical path:
                        # c = av[0:64] * (1/av[64]) broadcast over rows
                        for idx, ps_av in ((0, ps_av0), (1, ps_av1)):
                            av_sb = normpool.tile([DH + 1, 512], f32, tag="avs")
                            nc.vector.tensor_copy(av_sb, ps_av)
                            recip = normpool.tile([1, 512], f32, tag="recip")
                            nc.vector.reciprocal(recip, av_sb[DH:DH + 1])
                            r_dram = dram_pool.tile([1, 512], f32, tag="rd")
                            nc.sync.dma_start(r_dram, recip)
                            r_rep = normpool.tile([DH, 512], f32, tag="rrep")
                            nc.sync.dma_start(
                                r_rep, r_dram.to_broadcast([DH, 512]))
                            nc.vector.tensor_tensor(
                                cT_all[idx * DH:(idx + 1) * DH, hp, qs],
                                av_sb[0:DH], r_rep, mybir.AluOpType.mult)
                    # partial out-projection, delayed one q-block so the
                    # preceding normalize chain stays off the critical path
                    if 3 in phases:
                        for tb in (range((qb - 1) * 4, qb * 4) if qb > 0
                                   else ()):
                            for eb in range(2):
                                ps = ps_o_pool.tile([P, 512], mybir.dt.float32,
                                                    tag="pp")
                                for dc in range(EC):
                                    nc.tensor.matmul(
                                        ps, cT_all[:, dc, tb * P:(tb + 1) * P],
                                        wo_sb[:, dc, eb * 512:(eb + 1) * 512],
                                        start=(dc == 0), stop=(dc == EC - 1))
                                o_sb = evict.tile([P, 512], f32, tag="o")
                                nc.vector.tensor_copy(o_sb, ps)
                                nc.sync.dma_start(
                                    outp.ap()[tb * P:(tb + 1) * P,
                                              eb * 512:(eb + 1) * 512],
                                    o_sb)
                # trailing q-block's out-projection
                for tb in (range((QB - 1) * 4, QB * 4)
                           if (2 in phases and 3 in phases) else ()):
                    for eb in range(2):
                        ps = ps_o_pool.tile([P, 512], mybir.dt.float32,
                                            tag="pp")
                        for dc in range(EC):
                            nc.tensor.matmul(
                                ps, cT_all[:, dc, tb * P:(tb + 1) * P],
                                wo_sb[:, dc, eb * 512:(eb + 1) * 512],
                                start=(dc == 0), stop=(dc == EC - 1))
                        o_sb = evict.tile([P, 512], f32, tag="o")
                        nc.vector.tensor_copy(o_sb, ps)
                        nc.sync.dma_start(
                            outp.ap()[tb * P:(tb + 1) * P,
                                      eb * 512:(eb + 1) * 512],
                            o_sb)

    nc.finalize()
    return nc


def make_in_maps(x, Wq, bq, Wk, bk, Wv, bv, Wo, bo, mask):
    """Build the 8 per-core input dicts (host-side shard + transform)."""
    x = np.asarray(x, dtype=np.float32)
    Wq = np.asarray(Wq, dtype=np.float32)
    Wk = np.asarray(Wk, dtype=np.float32)
    Wv = np.asarray(Wv, dtype=np.float32)
    Wo = np.asarray(Wo, dtype=np.float32)
    bqf = np.asarray(bq, dtype=np.float32)
    bkf = np.asarray(bk, dtype=np.float32)
    mask = np.asarray(mask)

    scale = 1.0 / np.sqrt(np.float32(DH))
    # torch convention y = x @ W.T: feed W.T with d_in on axis 0
    WqT = np.ascontiguousarray(Wq.T) * scale        # [D, D], scale folded
    WkT = np.ascontiguousarray(Wk.T)
    WvT = np.ascontiguousarray(np.asarray(Wv, dtype=np.float32).T)
    WoT = np.ascontiguousarray(Wo.T)                # [D, D]

    # causal diag bias tiles from the mask input: tile j covers keys
    # [q0+128j, q0+128j+128) for query block [q0, q0+512); tril is
    # translation-invariant so build from q0 = S-512.
    q0 = S - 512
    m2 = mask.reshape(S, S)
    cm = np.empty((P, 4, 512), np.float32)
    for j in range(4):
        sub = m2[q0:q0 + 512, q0 + 128 * j:q0 + 128 * j + 128]  # [q, k]
        cm[:, j, :] = np.where(sub.T != 0, 1.0, 0.0)

    xTs = [np.ascontiguousarray(x[b].T) for b in range(B)]  # [D, S] each
    wslices = {}
    for g in range(2):
        cols = slice(g * E, (g + 1) * E)
        wslices[g] = {
            "wq": np.ascontiguousarray(WqT[:, cols]),
            "wk": np.ascontiguousarray(WkT[:, cols]),
            "wv": np.ascontiguousarray(WvT[:, cols]),
            "wo": np.ascontiguousarray(WoT[cols, :]),
            "bq": np.ascontiguousarray((bqf[cols] * scale).reshape(EC, P).T),
            "bk": np.ascontiguousarray(bkf[cols].reshape(EC, P).T),
        }
    in_maps = []
    for core in range(NCORES):
        b, g = divmod(core, 2)
        cols = slice(g * E, (g + 1) * E)
        in_maps.append({
            "xT": xTs[b],                                # [D, S]
            **wslices[g],
            "cm": cm,
        })
    return in_maps


def assemble_output(results, bv, bo, Wo):
    """Sum per-batch partials and add the bias correction."""
    bv = np.asarray(bv, dtype=np.float32)
    bo = np.asarray(bo, dtype=np.float32)
    Wo = np.asarray(Wo, dtype=np.float32)
    # context bias bv contributes bv @ Wo.T (attn rows sum to 1)
    corr = (bo + bv @ Wo.T).astype(np.float32)      # [D]
    out = np.empty((B, S, D), np.float32)
    for b in range(B):
        out[b] = results[2 * b]["outp"] + results[2 * b + 1]["outp"] + corr
    return out


def kernel(x, Wq, bq, Wk, bk, Wv, bv, Wo, bo, mask):
    from concourse.bass_utils import run_bass_kernel_spmd

    if "nc" not in _CACHE:
        _CACHE["nc"] = _build_nc()
    nc = _CACHE["nc"]
    in_maps = make_in_maps(x, Wq, bq, Wk, bk, Wv, bv, Wo, bo, mask)
    res = run_bass_kernel_spmd(nc, in_maps, core_ids=list(range(NCORES)))
    return assemble_output(res.results, bv, bo, Wo)
